# revision 6
# baseline (speedup 1.0000x reference)
"""Fused Trainium2 kernel for nn_InstDecoder (segment_reduce + bmm).

v6: layout/partition-utilization rewrite driven by trace + microbench.

Phase A (per-label sums): host sorts each shard's voxels by label (label 0
dropped), pads each label to a 128 multiple, and streams the result as
bf16 [128, 512]-column matmul chunks against a constant ones[128,1]
stationary (LDW of ONE column vs the old per-tile 65-column reload).
Each matmul emits per-voxel-tile column sums; outputs steer to PSUM
partitions {0,32,64,96} via tile_position so four matmuls share a bank
and one [97,512] drain. NTT (padded tile count) is data-derived at the
first kernel() call; the NEFF is cached per NTT. The per-label fold,
1/count, and [100,64]@Wk+bk glue run on the host, which knows the sort
boundaries. bf16 (not fp8): a segment sum of zero-centered values keeps
O(eps) relative error - fp8's 2% blew the 2e-2 gate.

Phase B (bmm + store): microbench showed HBM *stores* from a 100-row
SBUF tile run at 165 GB/s vs 318 GB/s from 128 rows (engine spread
follows partitions). So compute the TRANSPOSED output: out.T[vox,mask]
with mask_features chunks [64ch, 128vox] as the stationary and the
constant pkT[64,100] as the moving operand. Every store is then a full
128-partition [128,12800] transfer; the host un-transposes for free.

PE clock: HAM throttles the PE to 1.2 GHz until ~3.4us of sustained
activity, and re-throttles after any ~3.4us idle gap (the v5 trace ran
ALL matmuls cold). Both phases issue warm-up matmuls on a scratch PSUM
bank during the initial DMA fill and between real matmul bursts to hold
the 2.4 GHz clock.

Sharding: core i = (batch i//4, voxel shard i%4); cross-shard fold on
host between the two launches.
"""

import ml_dtypes
import numpy as np

BF16 = ml_dtypes.bfloat16

# ---- problem constants (hardcoded per contract) ----
B = 2
C = 64
KD = 64
D, H, W = 8, 256, 256
M = D * H * W            # 524288 voxels per batch
NUM_MASKS = 100
NSH = 4                  # voxel shards per batch
MSH = M // NSH           # 131072 voxels per core
NCORES = B * NSH

PA_NDMA = 6              # ft dma chunks
PA_WARM = 40             # phase-A initial warm-up matmuls
PA_PAD = 10              # keep-warm matmuls per chunk

# phase-B tiling: mf tiles [128, 8192] bf16 (2 voxel blocks of 8192)
PB_BLK = 16384           # voxels per tile
PB_NT = MSH // PB_BLK    # 8 tiles
PB_NMM = PB_BLK // 512   # 32 matmuls per tile
PB_WARM = 40             # phase-B initial warm-up matmuls
PB_PAD = 24              # keep-warm matmuls per tile

_STATE = {}
PROFILE = None


def _tile_context(nc):
    """TileContext whose kernel-tail drain splits its semaphore waits into
    one wait_ge instruction each - this container's walrus rejects CTRL
    instructions carrying more than a couple of sync waits."""
    import concourse.tile as tile
    from concourse.vector_clock import ScopedClock

    class _SplitDrainTC(tile.TileContext):
        def _drain_and_barrier(self, tick_clock, wait_clock):
            nc = self.nc
            drain_inst = nc.sync.drain()
            wait_clock.add_sem_waits(
                drain_inst.ins, ScopedClock({None: tick_clock.global_clock}))
            si = drain_inst.ins.sync_info
            waits = list(si.on_wait) if si and si.on_wait else []
            handles = {s.name: s for s in self.sems.allocated().values()}
            if waits:
                si.on_wait = []
                for w in waits:
                    nc.sync.wait_ge(handles[w.ant_name], w.wait_value)
            nc.all_engine_barrier()
            popped = nc._tile_sem_poison_stack.pop()
            assert popped is self._sem_poison
            nc.clear_and_free_semaphores(list(self.sems.allocated().values()))
            nc.all_engine_barrier()

    return _SplitDrainTC(nc)


def _split_excess_waits(nc, max_waits=1):
    """Move excess semaphore waits onto same-engine nops inserted before the
    offending instruction (monotonic sems make this equivalent)."""
    import bass_rust

    created = {}
    new_names = set()
    for bb in nc.main_func.blocks:
        for ins in bb.instructions:
            if ins.name in new_names:
                continue
            si = ins.sync_info
            if si and si.on_wait and len(si.on_wait) > max_waits:
                waits = list(si.on_wait)
                si.on_wait = waits[:max_waits]
                extra = waits[max_waits:]
                nops = []
                for k in range(0, len(extra), max_waits):
                    n = nc.engines[ins.engine].nop(nofuse=True)
                    n.ins.sync_info = bass_rust.SyncInfo(
                        on_wait=extra[k:k + max_waits], on_update=[])
                    nops.append(n.ins)
                    new_names.add(n.ins.name)
                created[ins.name] = nops
    if not created:
        return
    for bb in nc.main_func.blocks:
        out = []
        for ins in bb.instructions:
            if ins.name in new_names:
                continue
            if ins.name in created:
                out.extend(created[ins.name])
            out.append(ins)
        bb.instructions = out


def _build_phase_a(mm):
    """mm = number of 512-column matmuls (= padded voxel tiles / 8)."""
    import concourse.bass as bass
    import concourse.mybir as mybir

    f32 = mybir.dt.float32
    bf16 = mybir.dt.bfloat16
    gr = mm // 4
    nc = bass.Bass()
    ft = nc.declare_dram_parameter("ft", [128, mm * 512], bf16,
                                   isOutput=False)
    pa = nc.declare_dram_parameter("pa", [4, gr * 512], f32, isOutput=True)
    # chunk boundaries in matmul units
    base, rem = divmod(mm, PA_NDMA)
    sizes = [base + (1 if c < rem else 0) for c in range(PA_NDMA)]
    starts = np.concatenate([[0], np.cumsum(sizes)])
    with _tile_context(nc) as tc:
        with tc.tile_pool(name="const", bufs=1) as constp, \
             tc.tile_pool(name="ftp", bufs=3) as ftp, \
             tc.tile_pool(name="psa", bufs=7, space="PSUM") as psa, \
             tc.tile_pool(name="psw", bufs=1, space="PSUM") as psw:
            ones_t = constp.tile([128, 1], bf16)
            nc.vector.memset(ones_t[:], 1.0)
            wtile = constp.tile([128, 512], bf16)
            nc.vector.memset(wtile[:], 0.78125)
            wps = psw.tile([128, 512], f32)

            def warm(n):
                for _ in range(n):
                    nc.tensor.matmul(wps[:], lhsT=wtile[:, 0:128],
                                     rhs=wtile[:], start=True, stop=True,
                                     skip_group_check=True)

            dr = constp.tile([128, gr * 512], f32)
            warm(PA_WARM)
            for c in range(PA_NDMA):
                cmm = sizes[c]
                ftt = ftp.tile([128, cmm * 512], bf16, tag="ft")
                nc.sync.dma_start(
                    out=ftt[:], in_=ft[:, starts[c] * 512:starts[c + 1] * 512])
                for k in range(cmm):
                    i = starts[c] + k            # global matmul index
                    g, o = divmod(i, 4)
                    if o == 0:
                        ps = psa.tile([128, 512], f32, tag="ps")
                    nc.tensor.matmul(
                        ps[32 * o:32 * o + 1, :],
                        lhsT=ones_t[:],
                        rhs=ftt[:, 512 * k:512 * (k + 1)],
                        start=True, stop=True,
                        tile_position=(0, 32 * o),
                    )
                    if o == 3 or i == mm - 1:
                        eng = nc.vector.tensor_copy if g % 2 == 0 \
                            else nc.scalar.copy
                        eng(out=dr[0:97, 512 * g:512 * (g + 1)],
                            in_=ps[0:97, :])
                        if g == gr // 2 - 1:
                            nc.gpsimd.dma_start(
                                out=pa[:, 0:512 * (gr // 2)],
                                in_=dr[0:97:32, 0:512 * (gr // 2)])
                warm(PA_PAD)
            nc.scalar.dma_start(out=pa[:, 512 * (gr // 2):],
                                in_=dr[0:97:32, 512 * (gr // 2):])
    _split_excess_waits(nc)
    return nc


def _build_phase_b():
    import concourse.bass as bass
    import concourse.mybir as mybir

    f32 = mybir.dt.float32
    bf16 = mybir.dt.bfloat16
    nc = bass.Bass()
    pkt_d = nc.declare_dram_parameter("pkt", [128, 128], bf16,
                                      isOutput=False)
    mf = nc.declare_dram_parameter("mf", [PB_NT, 128, PB_BLK // 2], bf16,
                                   isOutput=False)
    om = nc.declare_dram_parameter("om", [PB_NT, 128, PB_BLK], bf16,
                                   isOutput=True)

    with _tile_context(nc) as tc:
        with tc.tile_pool(name="const", bufs=1) as constp, \
             tc.tile_pool(name="mfp", bufs=3) as mfp, \
             tc.tile_pool(name="obp", bufs=3) as obp, \
             tc.tile_pool(name="psb", bufs=6, space="PSUM") as psb, \
             tc.tile_pool(name="psw", bufs=1, space="PSUM") as psw:
            pkt = constp.tile([128, 128], bf16)
            nc.sync.dma_start(out=pkt[:], in_=pkt_d[:])
            wtile = constp.tile([128, 512], bf16)
            nc.vector.memset(wtile[:], 0.78125)
            wps = psw.tile([128, 512], f32)

            def warm(n):
                for _ in range(n):
                    nc.tensor.matmul(wps[:], lhsT=wtile[:, 0:128],
                                     rhs=wtile[:], start=True, stop=True,
                                     skip_group_check=True)

            warm(PB_WARM)
            for t in range(PB_NT):
                mft = mfp.tile([128, PB_BLK // 2], bf16, tag="mf")
                nc.sync.dma_start(out=mft[:], in_=mf[t])
                ob = obp.tile([128, PB_BLK], bf16, tag="ob")
                for v in range(PB_NMM):
                    b, c = divmod(v, PB_NMM // 2)
                    ps = psb.tile([128, 512], f32, tag="ps")
                    nc.tensor.matmul(
                        ps[:],
                        lhsT=pkt[64 * b:64 * (b + 1), :],
                        rhs=mft[64 * b:64 * (b + 1), 512 * c:512 * (c + 1)],
                        start=True, stop=True)
                    eng = nc.vector.tensor_copy if v % 2 == 0 \
                        else nc.scalar.copy
                    eng(out=ob[:, 8192 * b + 512 * c:8192 * b + 512 * (c + 1)],
                        in_=ps[:])
                steng = nc.scalar if t % 2 == 0 else nc.gpsimd
                steng.dma_start(out=om[t], in_=ob[:])
                warm(PB_PAD)
    _split_excess_waits(nc)
    return nc


def _get_state(mm):
    key = ("nc1", mm)
    if key not in _STATE:
        _STATE[key] = _build_phase_a(mm)
    if "nc2" not in _STATE:
        _STATE["nc2"] = _build_phase_b()
    return _STATE[key], _STATE["nc2"]


def _shard_layout(labs):
    """Sort info for one shard: per-label counts, padded tile offsets."""
    order = np.argsort(labs, kind="stable")
    slabs = labs[order]
    start1 = np.searchsorted(slabs, 1)      # drop label 0
    order = order[start1:]
    slabs = slabs[start1:]
    counts = np.bincount(slabs, minlength=NUM_MASKS + 1)[1:]  # [100]
    ntiles = (counts + 127) // 128
    tile_off = np.zeros(NUM_MASKS + 1, np.int64)
    np.cumsum(ntiles, out=tile_off[1:])
    return order, slabs, counts, tile_off


def _fill_shard(order, slabs, counts, tile_off, fsh_T, ntt, mm):
    cum = np.zeros(NUM_MASKS + 1, np.int64)
    np.cumsum(counts, out=cum[1:])
    within = np.arange(len(slabs)) - cum[slabs - 1]
    dest = tile_off[slabs - 1] * 128 + within
    buf = np.zeros((ntt * 128, C), np.float32)
    buf[dest] = fsh_T[order]
    ft_host = np.ascontiguousarray(
        buf.astype(BF16)
           .reshape(mm, 8, 128, C)
           .transpose(2, 0, 1, 3)
           .reshape(128, mm * 512))
    return ft_host


def kernel(features, mask_features, Wk, bk, init_masks):
    from concourse.bass_utils import run_bass_kernel_spmd

    features = np.asarray(features, dtype=np.float32)
    mask_features = np.asarray(mask_features, dtype=np.float32)
    Wk = np.ascontiguousarray(np.asarray(Wk, dtype=np.float32))
    bk = np.asarray(bk, dtype=np.float32)
    init_masks = np.asarray(init_masks)

    # ---- host-side sharding / layout prep ----
    feat = features.reshape(B, C, M)
    labsB = init_masks.reshape(B, M)
    mfr = mask_features.reshape(B, C, M)

    layouts = []
    for b in range(B):
        for s in range(NSH):
            sl = slice(s * MSH, (s + 1) * MSH)
            layouts.append(_shard_layout(labsB[b, sl]))
    ntt_max = max(int(lo[3][-1]) for lo in layouts)
    ntt = ((ntt_max + 31) // 32) * 32
    mm = ntt // 8
    gr = mm // 4

    nc1, nc2 = _get_state(mm)

    in_maps = []
    in_maps2 = []
    for b in range(B):
        fT = np.ascontiguousarray(feat[b].T)          # [M, C]
        for s in range(NSH):
            i = b * NSH + s
            sl = slice(s * MSH, (s + 1) * MSH)
            order, slabs, counts, tile_off = layouts[i]
            ft_host = _fill_shard(order, slabs, counts, tile_off,
                                  fT[sl], ntt, mm)
            in_maps.append({"ft": ft_host})
            mf_c = np.ascontiguousarray(
                mfr[b, :, sl].astype(BF16)
                   .reshape(C, PB_NT, 2, PB_BLK // 2)
                   .transpose(1, 2, 0, 3)
                   .reshape(PB_NT, 128, PB_BLK // 2))
            in_maps2.append({"mf": mf_c})

    trace = PROFILE is not None
    res1 = run_bass_kernel_spmd(nc1, in_maps, list(range(NCORES)),
                                trace=trace)
    if PROFILE is not None:
        PROFILE["phase1"] = res1.exec_time_ns

    # host glue: fold tile sums -> label sums, combine shards, apply
    # 1/count, Wk, bk -> duplicated pkT per batch
    for b in range(B):
        sums = np.zeros((NUM_MASKS, C), np.float32)
        cnts = np.zeros(NUM_MASKS, np.int64)
        for s in range(NSH):
            i = b * NSH + s
            pa = res1.results[i]["pa"]               # [4, gr*512]
            ts = pa.reshape(4, gr, 512).transpose(1, 0, 2) \
                   .reshape(gr * 4, 512)[:mm] \
                   .reshape(ntt, C)                  # per-tile sums
            counts, tile_off = layouts[i][2], layouts[i][3]
            for l in range(NUM_MASKS):
                t0, t1 = tile_off[l], tile_off[l + 1]
                if t1 > t0:
                    sums[l] += ts[t0:t1].sum(axis=0)
            cnts += counts
        inst = sums / np.maximum(cnts, 1)[:, None]   # [100, C]
        pkT = np.zeros((KD, 128), np.float32)        # 28 pad masks
        pkT[:, :NUM_MASKS] = (inst @ Wk + bk).T
        pkT2 = np.ascontiguousarray(
            np.concatenate([pkT, pkT], axis=0).astype(BF16))
        for s in range(NSH):
            in_maps2[b * NSH + s]["pkt"] = pkT2

    res = run_bass_kernel_spmd(nc2, in_maps2, list(range(NCORES)),
                               trace=trace)
    if PROFILE is not None:
        PROFILE["phase2"] = res.exec_time_ns

    out = np.empty((B, NUM_MASKS, M), np.float32)
    for i in range(NCORES):
        b, s = divmod(i, NSH)
        omr = res.results[i]["om"]          # [PB_NT, 128, PB_BLK]
        out[b, :, s * MSH:(s + 1) * MSH] = \
            omr[:, :NUM_MASKS, :].transpose(1, 0, 2).reshape(NUM_MASKS, MSH)
    return out.reshape(B, NUM_MASKS, D, H, W)


# revision 7
# speedup vs baseline: 1.0797x; 1.0797x over previous
"""Fused Trainium2 kernel for nn_InstDecoder (segment_reduce + bmm).

v6: layout/partition-utilization rewrite driven by trace + microbench.

Phase A (per-label sums): host sorts each shard's voxels by label (label 0
dropped), pads each label to a 128 multiple, and streams the result as
bf16 [128, 512]-column matmul chunks against a constant ones[128,1]
stationary (LDW of ONE column vs the old per-tile 65-column reload).
Each matmul emits per-voxel-tile column sums; outputs steer to PSUM
partitions {0,32,64,96} via tile_position so four matmuls share a bank
and one [97,512] drain. NTT (padded tile count) is data-derived at the
first kernel() call; the NEFF is cached per NTT. The per-label fold,
1/count, and [100,64]@Wk+bk glue run on the host, which knows the sort
boundaries. bf16 (not fp8): a segment sum of zero-centered values keeps
O(eps) relative error - fp8's 2% blew the 2e-2 gate.

Phase B (bmm + store): microbench showed HBM *stores* from a 100-row
SBUF tile run at 165 GB/s vs 318 GB/s from 128 rows (engine spread
follows partitions). So compute the TRANSPOSED output: out.T[vox,mask]
with mask_features chunks [64ch, 128vox] as the stationary and the
constant pkT[64,100] as the moving operand. Every store is then a full
128-partition [128,12800] transfer; the host un-transposes for free.

PE clock: HAM throttles the PE to 1.2 GHz until ~3.4us of sustained
activity, and re-throttles after any ~3.4us idle gap (the v5 trace ran
ALL matmuls cold). Both phases issue warm-up matmuls on a scratch PSUM
bank during the initial DMA fill and between real matmul bursts to hold
the 2.4 GHz clock.

Sharding: core i = (batch i//4, voxel shard i%4); cross-shard fold on
host between the two launches.
"""

import ml_dtypes
import numpy as np

BF16 = ml_dtypes.bfloat16

# ---- problem constants (hardcoded per contract) ----
B = 2
C = 64
KD = 64
D, H, W = 8, 256, 256
M = D * H * W            # 524288 voxels per batch
NUM_MASKS = 100
NSH = 4                  # voxel shards per batch
MSH = M // NSH           # 131072 voxels per core
NCORES = B * NSH

PA_NDMA = 6              # ft dma chunks
PA_WARM = 16             # phase-A initial warm-up matmuls
PA_PAD = 3               # keep-warm matmuls per chunk

# phase-B tiling: mf tiles [128, 8192] bf16 (2 voxel blocks of 8192)
PB_BLK = 16384           # voxels per tile
PB_NT = MSH // PB_BLK    # 8 tiles
PB_NMM = PB_BLK // 512   # 32 matmuls per tile
PB_WARM = 20             # phase-B initial warm-up matmuls
PB_PAD = 4               # keep-warm matmuls per tile

_STATE = {}
PROFILE = None


def _tile_context(nc):
    """TileContext whose kernel-tail drain splits its semaphore waits into
    one wait_ge instruction each - this container's walrus rejects CTRL
    instructions carrying more than a couple of sync waits."""
    import concourse.tile as tile
    from concourse.vector_clock import ScopedClock

    class _SplitDrainTC(tile.TileContext):
        def _drain_and_barrier(self, tick_clock, wait_clock):
            nc = self.nc
            drain_inst = nc.sync.drain()
            wait_clock.add_sem_waits(
                drain_inst.ins, ScopedClock({None: tick_clock.global_clock}))
            si = drain_inst.ins.sync_info
            waits = list(si.on_wait) if si and si.on_wait else []
            handles = {s.name: s for s in self.sems.allocated().values()}
            if waits:
                si.on_wait = []
                for w in waits:
                    nc.sync.wait_ge(handles[w.ant_name], w.wait_value)
            nc.all_engine_barrier()
            popped = nc._tile_sem_poison_stack.pop()
            assert popped is self._sem_poison
            nc.clear_and_free_semaphores(list(self.sems.allocated().values()))
            nc.all_engine_barrier()

    return _SplitDrainTC(nc)


def _split_excess_waits(nc, max_waits=1):
    """Move excess semaphore waits onto same-engine nops inserted before the
    offending instruction (monotonic sems make this equivalent)."""
    import bass_rust

    created = {}
    new_names = set()
    for bb in nc.main_func.blocks:
        for ins in bb.instructions:
            if ins.name in new_names:
                continue
            si = ins.sync_info
            if si and si.on_wait and len(si.on_wait) > max_waits:
                waits = list(si.on_wait)
                si.on_wait = waits[:max_waits]
                extra = waits[max_waits:]
                nops = []
                for k in range(0, len(extra), max_waits):
                    n = nc.engines[ins.engine].nop(nofuse=True)
                    n.ins.sync_info = bass_rust.SyncInfo(
                        on_wait=extra[k:k + max_waits], on_update=[])
                    nops.append(n.ins)
                    new_names.add(n.ins.name)
                created[ins.name] = nops
    if not created:
        return
    for bb in nc.main_func.blocks:
        out = []
        for ins in bb.instructions:
            if ins.name in new_names:
                continue
            if ins.name in created:
                out.extend(created[ins.name])
            out.append(ins)
        bb.instructions = out


def _build_phase_a(mm):
    """mm = number of 512-column matmuls (= padded voxel tiles / 8)."""
    import concourse.bass as bass
    import concourse.mybir as mybir

    f32 = mybir.dt.float32
    bf16 = mybir.dt.bfloat16
    gr = mm // 4
    nc = bass.Bass()
    ft = nc.declare_dram_parameter("ft", [128, mm * 512], bf16,
                                   isOutput=False)
    pa = nc.declare_dram_parameter("pa", [4, gr * 512], f32, isOutput=True)
    # chunk boundaries in matmul units
    base, rem = divmod(mm, PA_NDMA)
    sizes = [base + (1 if c < rem else 0) for c in range(PA_NDMA)]
    starts = np.concatenate([[0], np.cumsum(sizes)])
    with _tile_context(nc) as tc:
        with tc.tile_pool(name="const", bufs=1) as constp, \
             tc.tile_pool(name="ftp", bufs=3) as ftp, \
             tc.tile_pool(name="psa", bufs=7, space="PSUM") as psa, \
             tc.tile_pool(name="psw", bufs=1, space="PSUM") as psw:
            ones_t = constp.tile([128, 1], bf16)
            nc.vector.memset(ones_t[:], 1.0)
            wtile = constp.tile([128, 512], bf16)
            nc.vector.memset(wtile[:], 0.78125)
            wps = psw.tile([128, 512], f32)

            def warm(n):
                for _ in range(n):
                    nc.tensor.matmul(wps[:], lhsT=wtile[:, 0:128],
                                     rhs=wtile[:], start=True, stop=True,
                                     skip_group_check=True)

            dr = constp.tile([128, gr * 512], f32)
            warm(PA_WARM)
            for c in range(PA_NDMA):
                cmm = sizes[c]
                ftt = ftp.tile([128, cmm * 512], bf16, tag="ft")
                nc.sync.dma_start(
                    out=ftt[:], in_=ft[:, starts[c] * 512:starts[c + 1] * 512])
                for k in range(cmm):
                    i = starts[c] + k            # global matmul index
                    g, o = divmod(i, 4)
                    if o == 0:
                        ps = psa.tile([128, 512], f32, tag="ps")
                    nc.tensor.matmul(
                        ps[32 * o:32 * o + 1, :],
                        lhsT=ones_t[:],
                        rhs=ftt[:, 512 * k:512 * (k + 1)],
                        start=True, stop=True,
                        tile_position=(0, 32 * o),
                    )
                    if o == 3 or i == mm - 1:
                        eng = nc.vector.tensor_copy if g % 2 == 0 \
                            else nc.scalar.copy
                        eng(out=dr[0:97, 512 * g:512 * (g + 1)],
                            in_=ps[0:97, :])
                        if g == gr // 2 - 1:
                            nc.gpsimd.dma_start(
                                out=pa[:, 0:512 * (gr // 2)],
                                in_=dr[0:97:32, 0:512 * (gr // 2)])
                warm(PA_PAD)
            nc.scalar.dma_start(out=pa[:, 512 * (gr // 2):],
                                in_=dr[0:97:32, 512 * (gr // 2):])
    _split_excess_waits(nc)
    return nc


def _build_phase_b():
    import concourse.bass as bass
    import concourse.mybir as mybir

    f32 = mybir.dt.float32
    bf16 = mybir.dt.bfloat16
    nc = bass.Bass()
    pkt_d = nc.declare_dram_parameter("pkt", [128, 128], bf16,
                                      isOutput=False)
    mf = nc.declare_dram_parameter("mf", [PB_NT, 128, PB_BLK // 2], bf16,
                                   isOutput=False)
    om = nc.declare_dram_parameter("om", [PB_NT, 128, PB_BLK], bf16,
                                   isOutput=True)

    with _tile_context(nc) as tc:
        with tc.tile_pool(name="const", bufs=1) as constp, \
             tc.tile_pool(name="mfp", bufs=3) as mfp, \
             tc.tile_pool(name="obp", bufs=3) as obp, \
             tc.tile_pool(name="psb", bufs=6, space="PSUM") as psb, \
             tc.tile_pool(name="psw", bufs=1, space="PSUM") as psw:
            pkt = constp.tile([128, 128], bf16)
            nc.sync.dma_start(out=pkt[:], in_=pkt_d[:])
            wtile = constp.tile([128, 512], bf16)
            nc.vector.memset(wtile[:], 0.78125)
            wps = psw.tile([128, 512], f32)

            def warm(n):
                for _ in range(n):
                    nc.tensor.matmul(wps[:], lhsT=wtile[:, 0:128],
                                     rhs=wtile[:], start=True, stop=True,
                                     skip_group_check=True)

            warm(PB_WARM)
            for t in range(PB_NT):
                mft = mfp.tile([128, PB_BLK // 2], bf16, tag="mf")
                nc.sync.dma_start(out=mft[:], in_=mf[t])
                ob = obp.tile([128, PB_BLK], bf16, tag="ob")
                for v in range(PB_NMM):
                    b, c = divmod(v, PB_NMM // 2)
                    ps = psb.tile([128, 512], f32, tag="ps")
                    nc.tensor.matmul(
                        ps[:],
                        lhsT=pkt[64 * b:64 * (b + 1), :],
                        rhs=mft[64 * b:64 * (b + 1), 512 * c:512 * (c + 1)],
                        start=True, stop=True)
                    eng = nc.vector.tensor_copy if v % 2 == 0 \
                        else nc.scalar.copy
                    eng(out=ob[:, 8192 * b + 512 * c:8192 * b + 512 * (c + 1)],
                        in_=ps[:])
                steng = nc.scalar if t % 2 == 0 else nc.gpsimd
                steng.dma_start(out=om[t], in_=ob[:])
                warm(PB_PAD)
    _split_excess_waits(nc)
    return nc


def _get_state(mm):
    key = ("nc1", mm)
    if key not in _STATE:
        _STATE[key] = _build_phase_a(mm)
    if "nc2" not in _STATE:
        _STATE["nc2"] = _build_phase_b()
    return _STATE[key], _STATE["nc2"]


def _shard_layout(labs):
    """Sort info for one shard: per-label counts, padded tile offsets."""
    order = np.argsort(labs, kind="stable")
    slabs = labs[order]
    start1 = np.searchsorted(slabs, 1)      # drop label 0
    order = order[start1:]
    slabs = slabs[start1:]
    counts = np.bincount(slabs, minlength=NUM_MASKS + 1)[1:]  # [100]
    ntiles = (counts + 127) // 128
    tile_off = np.zeros(NUM_MASKS + 1, np.int64)
    np.cumsum(ntiles, out=tile_off[1:])
    return order, slabs, counts, tile_off


def _fill_shard(order, slabs, counts, tile_off, fsh_T, ntt, mm):
    cum = np.zeros(NUM_MASKS + 1, np.int64)
    np.cumsum(counts, out=cum[1:])
    within = np.arange(len(slabs)) - cum[slabs - 1]
    dest = tile_off[slabs - 1] * 128 + within
    buf = np.zeros((ntt * 128, C), np.float32)
    buf[dest] = fsh_T[order]
    ft_host = np.ascontiguousarray(
        buf.astype(BF16)
           .reshape(mm, 8, 128, C)
           .transpose(2, 0, 1, 3)
           .reshape(128, mm * 512))
    return ft_host


def kernel(features, mask_features, Wk, bk, init_masks):
    from concourse.bass_utils import run_bass_kernel_spmd

    features = np.asarray(features, dtype=np.float32)
    mask_features = np.asarray(mask_features, dtype=np.float32)
    Wk = np.ascontiguousarray(np.asarray(Wk, dtype=np.float32))
    bk = np.asarray(bk, dtype=np.float32)
    init_masks = np.asarray(init_masks)

    # ---- host-side sharding / layout prep ----
    feat = features.reshape(B, C, M)
    labsB = init_masks.reshape(B, M)
    mfr = mask_features.reshape(B, C, M)

    layouts = []
    for b in range(B):
        for s in range(NSH):
            sl = slice(s * MSH, (s + 1) * MSH)
            layouts.append(_shard_layout(labsB[b, sl]))
    ntt_max = max(int(lo[3][-1]) for lo in layouts)
    ntt = ((ntt_max + 31) // 32) * 32
    mm = ntt // 8
    gr = mm // 4

    nc1, nc2 = _get_state(mm)

    in_maps = []
    in_maps2 = []
    for b in range(B):
        fT = np.ascontiguousarray(feat[b].T)          # [M, C]
        for s in range(NSH):
            i = b * NSH + s
            sl = slice(s * MSH, (s + 1) * MSH)
            order, slabs, counts, tile_off = layouts[i]
            ft_host = _fill_shard(order, slabs, counts, tile_off,
                                  fT[sl], ntt, mm)
            in_maps.append({"ft": ft_host})
            mf_c = np.ascontiguousarray(
                mfr[b, :, sl].astype(BF16)
                   .reshape(C, PB_NT, 2, PB_BLK // 2)
                   .transpose(1, 2, 0, 3)
                   .reshape(PB_NT, 128, PB_BLK // 2))
            in_maps2.append({"mf": mf_c})

    trace = PROFILE is not None
    res1 = run_bass_kernel_spmd(nc1, in_maps, list(range(NCORES)),
                                trace=trace)
    if PROFILE is not None:
        PROFILE["phase1"] = res1.exec_time_ns

    # host glue: fold tile sums -> label sums, combine shards, apply
    # 1/count, Wk, bk -> duplicated pkT per batch
    for b in range(B):
        sums = np.zeros((NUM_MASKS, C), np.float32)
        cnts = np.zeros(NUM_MASKS, np.int64)
        for s in range(NSH):
            i = b * NSH + s
            pa = res1.results[i]["pa"]               # [4, gr*512]
            ts = pa.reshape(4, gr, 512).transpose(1, 0, 2) \
                   .reshape(gr * 4, 512)[:mm] \
                   .reshape(ntt, C)                  # per-tile sums
            counts, tile_off = layouts[i][2], layouts[i][3]
            for l in range(NUM_MASKS):
                t0, t1 = tile_off[l], tile_off[l + 1]
                if t1 > t0:
                    sums[l] += ts[t0:t1].sum(axis=0)
            cnts += counts
        inst = sums / np.maximum(cnts, 1)[:, None]   # [100, C]
        pkT = np.zeros((KD, 128), np.float32)        # 28 pad masks
        pkT[:, :NUM_MASKS] = (inst @ Wk + bk).T
        pkT2 = np.ascontiguousarray(
            np.concatenate([pkT, pkT], axis=0).astype(BF16))
        for s in range(NSH):
            in_maps2[b * NSH + s]["pkt"] = pkT2

    res = run_bass_kernel_spmd(nc2, in_maps2, list(range(NCORES)),
                               trace=trace)
    if PROFILE is not None:
        PROFILE["phase2"] = res.exec_time_ns

    out = np.empty((B, NUM_MASKS, M), np.float32)
    for i in range(NCORES):
        b, s = divmod(i, NSH)
        omr = res.results[i]["om"]          # [PB_NT, 128, PB_BLK]
        out[b, :, s * MSH:(s + 1) * MSH] = \
            omr[:, :NUM_MASKS, :].transpose(1, 0, 2).reshape(NUM_MASKS, MSH)
    return out.reshape(B, NUM_MASKS, D, H, W)
